# revision 34
# baseline (speedup 1.0000x reference)
"""DressedQuantumNet on 8 TRN2 NeuronCores (pure data parallel).

Math: pre-net angles th = X @ pre_w.T + pre_b.  The quantum circuit after
the batch-dependent RY(th) layer is a FIXED unitary V (it only depends on
q_weights); the whole network collapses to an 81-coefficient multilinear
polynomial in v_w = [1, sin th_w, cos th_w]:

  out_k = sum_{m in 3^4} T_k[m] * prod_w v_w[m_w]

T_k is precomputed on host (tiny); the device computes the [B,512]@[512,4]
matmul, sin/cos, and the batched contraction.

v3 device layout (per core, batch on SBUF partitions):
  - X shipped fp16-only (rel err ~1.2e-3 vs the 2e-2 gate); host
    pre-transposed 1 MiB slabs; slabs alternate sync/scalar HWDGE rings.
  - 4 accumulating matmuls per 128-row tile -> angles [P, sg, 4] in PSUM.
  - per-super quantum stage, software-pipelined as  mm(k) | B(k-1) | A(k)
    so the DVE queue never head-of-line blocks on DMA:
      A(k): angles -> sin arguments in "turns" (3 ops: scalar_tensor_tensor
            + magic-number rint + sub), Sin activation with scale=2pi.
      B(k): pair tables (8-wide, no ones-row), tq = T8*w23, 2x-mode
            tree-adds instead of 1x tensor_reduce, then the w01 side.
  - all quantum-stage tensors fp16 with 4B-aligned power-of-2 layouts.
"""

from contextlib import ExitStack

import numpy as np

import concourse.bass as bass
import concourse.bacc as bacc_mod
import concourse.mybir as mybir
from concourse.bass_utils import run_bass_kernel_spmd
from concourse.tile import TileContext

N_CORES = 8
B_TOTAL = 65536
F_IN = 512
ROWS = B_TOTAL // N_CORES   # 8192 rows per core
P = 128
N_TILES = ROWS // P         # 64 row-tiles

F32 = mybir.dt.float32
FP16 = mybir.dt.float16
PI = float(np.pi)
MAGIC = float(1.5 * 2 ** 23)

N_QUBITS, VAR_DEPTH = 4, 3

# DMA slab sizes (tiles) and quantum-stage super-group sizes (tiles).
DMA_SCHED = [2, 6, 8, 8, 8, 8, 8, 8, 8]
SUPERS = [2, 6, 24, 16, 8, 8]
assert sum(DMA_SCHED) == N_TILES and sum(SUPERS) == N_TILES


# ----------------------------------------------------------------- host math
def _gate_1q(g, w):
    ops = [np.eye(2, dtype=complex)] * N_QUBITS
    ops[w] = g
    U = ops[0]
    for i in range(1, N_QUBITS):
        U = np.kron(U, ops[i])
    return U


def _bit(i, w):  # wire 0 = most significant
    return (i >> (N_QUBITS - 1 - w)) & 1


def _cnot(c, t):
    M = np.zeros((16, 16), dtype=complex)
    for i in range(16):
        j = i ^ (1 << (N_QUBITS - 1 - t)) if _bit(i, c) else i
        M[j, i] = 1.0
    return M


def _ry(theta):
    c, s = np.cos(theta / 2), np.sin(theta / 2)
    return np.array([[c, -s], [s, c]], dtype=complex)


def _rz(theta):
    ph = np.exp(1j * theta / 2)
    return np.array([[np.conj(ph), 0], [0, ph]], dtype=complex)


def _fixed_unitary(qw):
    V = np.eye(16, dtype=complex)

    def app(Gm):
        nonlocal V
        V = Gm @ V

    def entangle():
        app(_cnot(0, 1)); app(_cnot(2, 3)); app(_cnot(1, 2))

    for k in range(VAR_DEPTH):
        entangle()
        for w in range(N_QUBITS):
            app(_gate_1q(_ry(qw[k, w]), w))
        for w in range(N_QUBITS):
            app(_gate_1q(_rz(qw[k, w]), w))
    for k in range(VAR_DEPTH):
        entangle()
        for w in range(N_QUBITS):
            app(_gate_1q(_ry(qw[k, w]), w))
        for w in range(N_QUBITS):
            app(_gate_1q(_rz(qw[3 + k, w]), w))
    entangle()
    return V


def _build_T(q_weights, post_w, post_b):
    """[2, 81] coefficients; post_b folded into the constant term."""
    V = _fixed_unitary(np.asarray(q_weights, dtype=np.float64))
    E = np.zeros((3, 2, 2))
    E[0] = [[0.5, 0.0], [0.0, 0.5]]
    E[1] = [[-0.5, 0.0], [0.0, 0.5]]
    E[2] = [[0.0, 0.5], [0.5, 0.0]]
    Ts = []
    for k in range(2):
        C = np.zeros((16, 16), dtype=complex)
        for w in range(N_QUBITS):
            z = np.array([1.0 - 2.0 * _bit(i, w) for i in range(16)])
            C += post_w[k, w] * (V.conj().T @ np.diag(z) @ V)
        A = C.real.reshape([2] * 8)
        T = np.einsum("abcdefgh,iae,jbf,kcg,ldh->ijkl", A, E, E, E, E)
        T = T.reshape(81).copy()
        T[0] += post_b[k]
        Ts.append(T)
    return np.stack(Ts).astype(np.float32)  # [2, 81]


# ------------------------------------------------------------- device kernel
def build_bass(rows=ROWS):
    n_tiles = rows // P
    if rows == ROWS:
        dma_sched, supers = DMA_SCHED, SUPERS
    else:
        dma_sched = []
        while sum(dma_sched) < n_tiles:
            dma_sched.append(min(8, n_tiles - sum(dma_sched)))
        supers = list(dma_sched)
    dma_offs = [0]
    for s in dma_sched:
        dma_offs.append(dma_offs[-1] + s)
    sup_offs = [0]
    for s in supers:
        sup_offs.append(sup_offs[-1] + s)
    n_sup = len(supers)

    nc = bacc_mod.Bacc(None, target_bir_lowering=False)
    # host-packed flat: concatenation of per-slab [P, 4, gb] fp16 blocks
    ht_d = nc.dram_tensor("htp", [rows * 4 * P], FP16, kind="ExternalInput")
    whl_d = nc.dram_tensor("whl", [P, 16], FP16, kind="ExternalInput")
    bi_d = nc.dram_tensor("biad", [P, 2, 4], F32, kind="ExternalInput")
    t8_d = nc.dram_tensor("t8c", [P, 18, 8], FP16, kind="ExternalInput")
    t0_d = nc.dram_tensor("t0c", [P, 18], FP16, kind="ExternalInput")
    # out_dev[p, t, k] = out[t*128 + p, k]; host unscrambles
    out_d = nc.dram_tensor("out", [P, n_tiles, 2], F32, kind="ExternalOutput")

    with TileContext(nc) as tc, ExitStack() as ctx:
        const = ctx.enter_context(tc.tile_pool(name="const", bufs=1))
        # dummy activation fed by a memset tile: forces the Sin ACT table
        # load to start immediately, overlapping the input DMA instead of
        # sitting on the first super's critical path
        wsrc = const.tile([P, 2], F32)
        nc.vector.memset(wsrc, 0.25)
        warm = const.tile([P, 2], FP16)
        nc.scalar.activation(warm, wsrc, mybir.ActivationFunctionType.Sin)
        # const tiles; DMAs interleaved into the sync ring after the first
        # slabs (see below) so slab0/1 data starts flowing first
        whl = const.tile([P, 16], FP16)
        bia = const.tile([P, 2, 4], F32)
        t8 = const.tile([P, 18, 8], FP16)
        t0 = const.tile([P, 18], FP16)

        xp = ctx.enter_context(tc.tile_pool(name="xin", bufs=9))
        angp = ctx.enter_context(tc.tile_pool(name="angp", bufs=3, space="PSUM"))
        stg = ctx.enter_context(tc.tile_pool(name="stg", bufs=3))
        scr = ctx.enter_context(tc.tile_pool(name="scr", bufs=3))
        tqp = ctx.enter_context(tc.tile_pool(name="tq", bufs=2))
        rp = ctx.enter_context(tc.tile_pool(name="res", bufs=3))

        # all input-slab DMAs issued up front; early slabs on the sync ring
        # (the scalar ring is busy with the ACT table load at t=0); const
        # DMAs slot in behind the first two slab issues
        slabs = []
        n_sync = (len(dma_sched) + 1) // 2
        for gi, g_tiles in enumerate(dma_sched):
            gb = g_tiles * P
            base = dma_offs[gi] * P * 4
            ht_sb = xp.tile([P, 4, gb], FP16, tag="ht")
            eng = nc.sync if gi < n_sync else nc.scalar
            eng.dma_start(
                ht_sb,
                ht_d[base * P:(base + 4 * gb) * P].rearrange(
                    "(p k b) -> p k b", p=P, k=4),
            )
            slabs.append(ht_sb)
            if gi == 1:
                nc.sync.dma_start(whl, whl_d[:])
                nc.sync.dma_start(bia, bi_d[:])
            elif gi == 2:
                nc.sync.dma_start(t8, t8_d[:])
                nc.sync.dma_start(t0, t0_d[:])

        def ht_chunk(t, k):
            gi = 0
            while dma_offs[gi + 1] <= t:
                gi += 1
            bs = (t - dma_offs[gi]) * P
            return slabs[gi][:, k, bs:bs + P]

        state = [None] * n_sup  # per-super (sg, vv) for stage B

        def emit_mm(si):
            """angles in turns (whl pre-scaled by 1/2pi): ang[p,g,w]"""
            sg = supers[si]
            ang = angp.tile([P, sg, 4], F32)
            for lt in range(sg):
                for k in range(4):
                    nc.tensor.matmul(
                        ang[:, lt, :],
                        ht_chunk(sup_offs[si] + lt, k),
                        whl[:, 4 * k:4 * k + 4],
                        start=(k == 0), stop=(k == 3),
                    )
            return ang

        def emit_A(si, ang):
            """angles -> pair tables' sin/cos slots (fp16)."""
            sg = supers[si]
            # m = th in turns + biad (cos row gets +1/4 turn)
            m = scr.tile([P, sg, 2, 4], F32, tag="m")
            nc.vector.tensor_add(
                m, ang.unsqueeze(2).broadcast_to([P, sg, 2, 4]),
                bia.unsqueeze(1).broadcast_to([P, sg, 2, 4]),
            )
            # range wrap on the (otherwise idle) GpSimd engine, fp32 only:
            # r = rint(m) via the magic lattice, s = m - r in [-0.5, 0.5]
            r = scr.tile([P, sg, 2, 4], F32, tag="r")
            nc.gpsimd.tensor_scalar(
                r, m, MAGIC, MAGIC,
                op0=mybir.AluOpType.add, op1=mybir.AluOpType.subtract,
            )
            s = scr.tile([P, sg, 2, 4], F32, tag="s")
            nc.gpsimd.tensor_sub(s, m, r)
            # two ACTs write w-major straight into the pair tables' first 4
            # slots: wb[., X, 0:4] = [s_hi, c_hi, s_lo, c_lo] for pair X
            wb = stg.tile([P, sg, 2, 8], FP16, tag="wb")
            for X in (0, 1):
                nc.scalar.activation(
                    wb[:, :, X, 0:4].rearrange("p g (w r) -> p g w r", w=2),
                    s[:, :, :, 2 * X:2 * X + 2].rearrange("p g r w -> p g w r"),
                    mybir.ActivationFunctionType.Sin, scale=2.0 * PI,
                )
            state[si] = wb

        def emit_B(si):
            """pair tables + contraction for super si."""
            sg = supers[si]
            wb = state[si]
            # product slots 4:8 = [s_hi; c_hi] x [s_lo, c_lo]
            for X in (0, 1):
                nc.vector.tensor_mul(
                    wb[:, :, X, 4:8].rearrange("p g (a b) -> p g a b", a=2),
                    wb[:, :, X, 0:2].unsqueeze(3).broadcast_to([P, sg, 2, 2]),
                    wb[:, :, X, 2:4].unsqueeze(2).broadcast_to([P, sg, 2, 2]),
                )
            w01 = wb[:, :, 0, :]
            w23 = wb[:, :, 1, :]
            # tq[p,g,(k a),b] = T8[(k a), b] * w23[b]; 2x tree-add over b
            tq = tqp.tile([P, sg, 18, 8], FP16, tag="tq")
            nc.vector.tensor_mul(
                tq,
                w23.unsqueeze(2).broadcast_to([P, sg, 18, 8]),
                t8.unsqueeze(1).broadcast_to([P, sg, 18, 8]),
            )
            lq1 = tqp.tile([P, sg, 18, 4], FP16, tag="lq1")
            nc.vector.tensor_add(lq1, tq[:, :, :, 0:4], tq[:, :, :, 4:8])
            lq2 = tqp.tile([P, sg, 18, 2], FP16, tag="lq2")
            nc.vector.tensor_add(lq2, lq1[:, :, :, 0:2], lq1[:, :, :, 2:4])
            qk = stg.tile([P, sg, 18], FP16, tag="qk")
            nc.vector.tensor_add(qk, lq2[:, :, :, 0], lq2[:, :, :, 1])
            # qkf in fp32 so the GpSimd tail never touches fp16
            qkf = stg.tile([P, sg, 18], F32, tag="qkf")
            nc.vector.tensor_add(
                qkf, qk, t0.unsqueeze(1).broadcast_to([P, sg, 18])
            )
            qkv = qkf.rearrange("p g (k a) -> p g k a", k=2)
            sk = stg.tile([P, sg, 2, 8], F32, tag="sk")
            nc.vector.tensor_mul(
                sk, qkv[:, :, :, 0:8],
                w01.unsqueeze(2).broadcast_to([P, sg, 2, 8]),
            )
            # tree + a0 term on GpSimd (fp32), freeing the DVE queue
            m1 = scr.tile([P, sg, 2, 4], F32, tag="m1")
            nc.gpsimd.tensor_add(m1, sk[:, :, :, 0:4], sk[:, :, :, 4:8])
            m2 = scr.tile([P, sg, 2, 2], F32, tag="m2")
            nc.gpsimd.tensor_add(m2, m1[:, :, :, 0:2], m1[:, :, :, 2:4])
            m3 = scr.tile([P, sg, 2], F32, tag="m3")
            nc.gpsimd.tensor_add(m3, m2[:, :, :, 0], m2[:, :, :, 1])
            t0g = sup_offs[si]
            ro = rp.tile([P, sg, 2], F32, tag="ro")
            nc.gpsimd.tensor_add(ro, m3, qkv[:, :, :, 8])
            # stream this super's slice out now; hides the ~2us DMA
            # completion latency behind later supers' compute
            nc.sync.dma_start(out_d[:, t0g:t0g + sg, :], ro)

        # software pipeline: mm(k) | B(k-1) | A(k)
        angs = [None] * n_sup
        for si in range(n_sup):
            angs[si] = emit_mm(si)
            if si >= 1:
                emit_B(si - 1)
            emit_A(si, angs[si])
        emit_B(n_sup - 1)

    nc.finalize()
    return nc


_NC_CACHE = {}


def _get_nc(rows=ROWS):
    if rows not in _NC_CACHE:
        _NC_CACHE[rows] = build_bass(rows=rows)
    return _NC_CACHE[rows]


def _host_consts(pre_w, pre_b, q_weights, post_w, post_b):
    pre_w = np.asarray(pre_w, dtype=np.float32) / (2.0 * np.pi)
    wh = pre_w.astype(np.float16)
    # whl[f_loc, 4k + j] = W[j, 128k + f_loc] / 2pi  (fp16, "turns")
    whl = np.zeros((P, 16), dtype=np.float16)
    for k in range(4):
        whl[:, 4 * k:4 * k + 4] = wh.T[P * k:P * (k + 1)]
    T = _build_T(
        np.asarray(q_weights, np.float64),
        np.asarray(post_w, np.float64),
        np.asarray(post_b, np.float64),
    ).reshape(2, 9, 9)  # [k, a, b] in basis [1, s_lo, c_lo, s_hi, ...]
    # device slot order [s_hi, c_hi, s_lo, c_lo, shsl, shcl, chsl, chcl, 1]
    perm = [3, 6, 1, 2, 4, 5, 7, 8, 0]
    Tk = T[:, perm][:, :, perm]
    t8c = np.broadcast_to(
        Tk[:, :, 0:8].reshape(18, 8).astype(np.float16), (P, 18, 8)).copy()
    t0c = np.broadcast_to(
        Tk[:, :, 8].reshape(18).astype(np.float16), (P, 18)).copy()
    pb = np.asarray(pre_b, np.float64)
    # biad = (pre_b + [0, pi/2]) / 2pi  (sin row, cos row), in turns
    b2 = (np.stack([pb, pb + 0.5 * np.pi]) / (2.0 * np.pi)).astype(np.float32)
    biad = np.broadcast_to(b2, (P, 2, 4)).copy()
    return {
        "whl": np.ascontiguousarray(whl),
        "biad": np.ascontiguousarray(biad),
        "t8c": np.ascontiguousarray(t8c),
        "t0c": np.ascontiguousarray(t0c),
    }


def _split_transpose(x):
    """x [ROWS, F] f32 -> flat fp16: concatenated per-slab [P, 4, gb] packs,
    pack[p, k, b] = val[slab_row0 + b, 128*k + p]."""
    rows = x.shape[0]
    if rows == ROWS:
        sched = DMA_SCHED
    else:
        sched = []
        while sum(sched) < rows // P:
            sched.append(min(8, rows // P - sum(sched)))
    h = x.astype(np.float16)
    parts = []
    r0 = 0
    for s in sched:
        gb = s * P
        blk = h[r0:r0 + gb].reshape(gb, 4, P).transpose(2, 1, 0)
        parts.append(np.ascontiguousarray(blk).reshape(-1))
        r0 += gb
    return np.concatenate(parts)


def run(input_features, pre_w, pre_b, q_weights, post_w, post_b, **spmd_kwargs):
    x = np.asarray(input_features, dtype=np.float32)
    assert x.shape == (B_TOTAL, F_IN), x.shape
    consts = _host_consts(pre_w, pre_b, q_weights, post_w, post_b)
    in_maps = []
    for c in range(N_CORES):
        ht = _split_transpose(x[c * ROWS:(c + 1) * ROWS])
        in_maps.append(dict(consts, htp=ht))
    nc = _get_nc()
    r = run_bass_kernel_spmd(nc, in_maps, core_ids=list(range(N_CORES)), **spmd_kwargs)
    # out_dev[p, t, k] -> out[t*128 + p, k]
    out = np.concatenate(
        [r.results[c]["out"].transpose(1, 0, 2).reshape(ROWS, 2) for c in range(N_CORES)],
        axis=0,
    )
    return out.astype(np.float32), r


def kernel(input_features, pre_w, pre_b, q_weights, post_w, post_b):
    out, _ = run(input_features, pre_w, pre_b, q_weights, post_w, post_b)
    return out


# revision 35
# speedup vs baseline: 1.0133x; 1.0133x over previous
"""DressedQuantumNet on 8 TRN2 NeuronCores (pure data parallel).

Math: pre-net angles th = X @ pre_w.T + pre_b.  The quantum circuit after
the batch-dependent RY(th) layer is a FIXED unitary V (it only depends on
q_weights); the whole network collapses to an 81-coefficient multilinear
polynomial in v_w = [1, sin th_w, cos th_w]:

  out_k = sum_{m in 3^4} T_k[m] * prod_w v_w[m_w]

T_k is precomputed on host (tiny); the device computes the [B,512]@[512,4]
matmul, sin/cos, and the batched contraction.

v3 device layout (per core, batch on SBUF partitions):
  - X shipped fp16-only (rel err ~1.2e-3 vs the 2e-2 gate); host
    pre-transposed 1 MiB slabs; slabs alternate sync/scalar HWDGE rings.
  - 4 accumulating matmuls per 128-row tile -> angles [P, sg, 4] in PSUM.
  - per-super quantum stage, software-pipelined as  mm(k) | B(k-1) | A(k)
    so the DVE queue never head-of-line blocks on DMA:
      A(k): angles -> sin arguments in "turns" (3 ops: scalar_tensor_tensor
            + magic-number rint + sub), Sin activation with scale=2pi.
      B(k): pair tables (8-wide, no ones-row), tq = T8*w23, 2x-mode
            tree-adds instead of 1x tensor_reduce, then the w01 side.
  - all quantum-stage tensors fp16 with 4B-aligned power-of-2 layouts.
"""

from contextlib import ExitStack

import numpy as np

import concourse.bass as bass
import concourse.bacc as bacc_mod
import concourse.mybir as mybir
from concourse.bass_utils import run_bass_kernel_spmd
from concourse.tile import TileContext

N_CORES = 8
B_TOTAL = 65536
F_IN = 512
ROWS = B_TOTAL // N_CORES   # 8192 rows per core
P = 128
N_TILES = ROWS // P         # 64 row-tiles

F32 = mybir.dt.float32
FP16 = mybir.dt.float16
PI = float(np.pi)
MAGIC = float(1.5 * 2 ** 23)

N_QUBITS, VAR_DEPTH = 4, 3

# DMA slab sizes (tiles) and quantum-stage super-group sizes (tiles).
DMA_SCHED = [2, 6, 8, 8, 8, 8, 8, 8, 8]
SUPERS = [2, 6, 24, 16, 8, 8]
assert sum(DMA_SCHED) == N_TILES and sum(SUPERS) == N_TILES


# ----------------------------------------------------------------- host math
def _gate_1q(g, w):
    ops = [np.eye(2, dtype=complex)] * N_QUBITS
    ops[w] = g
    U = ops[0]
    for i in range(1, N_QUBITS):
        U = np.kron(U, ops[i])
    return U


def _bit(i, w):  # wire 0 = most significant
    return (i >> (N_QUBITS - 1 - w)) & 1


def _cnot(c, t):
    M = np.zeros((16, 16), dtype=complex)
    for i in range(16):
        j = i ^ (1 << (N_QUBITS - 1 - t)) if _bit(i, c) else i
        M[j, i] = 1.0
    return M


def _ry(theta):
    c, s = np.cos(theta / 2), np.sin(theta / 2)
    return np.array([[c, -s], [s, c]], dtype=complex)


def _rz(theta):
    ph = np.exp(1j * theta / 2)
    return np.array([[np.conj(ph), 0], [0, ph]], dtype=complex)


def _fixed_unitary(qw):
    V = np.eye(16, dtype=complex)

    def app(Gm):
        nonlocal V
        V = Gm @ V

    def entangle():
        app(_cnot(0, 1)); app(_cnot(2, 3)); app(_cnot(1, 2))

    for k in range(VAR_DEPTH):
        entangle()
        for w in range(N_QUBITS):
            app(_gate_1q(_ry(qw[k, w]), w))
        for w in range(N_QUBITS):
            app(_gate_1q(_rz(qw[k, w]), w))
    for k in range(VAR_DEPTH):
        entangle()
        for w in range(N_QUBITS):
            app(_gate_1q(_ry(qw[k, w]), w))
        for w in range(N_QUBITS):
            app(_gate_1q(_rz(qw[3 + k, w]), w))
    entangle()
    return V


def _build_T(q_weights, post_w, post_b):
    """[2, 81] coefficients; post_b folded into the constant term."""
    V = _fixed_unitary(np.asarray(q_weights, dtype=np.float64))
    E = np.zeros((3, 2, 2))
    E[0] = [[0.5, 0.0], [0.0, 0.5]]
    E[1] = [[-0.5, 0.0], [0.0, 0.5]]
    E[2] = [[0.0, 0.5], [0.5, 0.0]]
    Ts = []
    for k in range(2):
        C = np.zeros((16, 16), dtype=complex)
        for w in range(N_QUBITS):
            z = np.array([1.0 - 2.0 * _bit(i, w) for i in range(16)])
            C += post_w[k, w] * (V.conj().T @ np.diag(z) @ V)
        A = C.real.reshape([2] * 8)
        T = np.einsum("abcdefgh,iae,jbf,kcg,ldh->ijkl", A, E, E, E, E)
        T = T.reshape(81).copy()
        T[0] += post_b[k]
        Ts.append(T)
    return np.stack(Ts).astype(np.float32)  # [2, 81]


# ------------------------------------------------------------- device kernel
def build_bass(rows=ROWS):
    n_tiles = rows // P
    if rows == ROWS:
        dma_sched, supers = DMA_SCHED, SUPERS
    else:
        dma_sched = []
        while sum(dma_sched) < n_tiles:
            dma_sched.append(min(8, n_tiles - sum(dma_sched)))
        supers = list(dma_sched)
    dma_offs = [0]
    for s in dma_sched:
        dma_offs.append(dma_offs[-1] + s)
    sup_offs = [0]
    for s in supers:
        sup_offs.append(sup_offs[-1] + s)
    n_sup = len(supers)

    nc = bacc_mod.Bacc(None, target_bir_lowering=False)
    # host-packed flat: concatenation of per-slab [P, 4, gb] fp16 blocks
    ht_d = nc.dram_tensor("htp", [rows * 4 * P], FP16, kind="ExternalInput")
    whl_d = nc.dram_tensor("whl", [P, 16], FP16, kind="ExternalInput")
    bi_d = nc.dram_tensor("biad", [P, 2, 4], F32, kind="ExternalInput")
    t8_d = nc.dram_tensor("t8c", [P, 18, 8], FP16, kind="ExternalInput")
    t0_d = nc.dram_tensor("t0c", [P, 18], FP16, kind="ExternalInput")
    # out_dev[p, t, k] = out[t*128 + p, k]; host unscrambles
    out_d = nc.dram_tensor("out", [P, n_tiles, 2], F32, kind="ExternalOutput")

    with TileContext(nc) as tc, ExitStack() as ctx:
        const = ctx.enter_context(tc.tile_pool(name="const", bufs=1))
        # dummy activation fed by a memset tile: forces the Sin ACT table
        # load to start immediately, overlapping the input DMA instead of
        # sitting on the first super's critical path
        wsrc = const.tile([P, 2], F32)
        nc.vector.memset(wsrc, 0.25)
        warm = const.tile([P, 2], FP16)
        nc.scalar.activation(warm, wsrc, mybir.ActivationFunctionType.Sin)
        # const tiles; DMAs interleaved into the sync ring after the first
        # slabs (see below) so slab0/1 data starts flowing first
        whl = const.tile([P, 16], FP16)
        bia = const.tile([P, 2, 4], F32)
        t8 = const.tile([P, 18, 8], FP16)
        t0 = const.tile([P, 18], FP16)

        xp = ctx.enter_context(tc.tile_pool(name="xin", bufs=9))
        angp = ctx.enter_context(tc.tile_pool(name="angp", bufs=3, space="PSUM"))
        stg = ctx.enter_context(tc.tile_pool(name="stg", bufs=3))
        scr = ctx.enter_context(tc.tile_pool(name="scr", bufs=3))
        tqp = ctx.enter_context(tc.tile_pool(name="tq", bufs=2))
        rp = ctx.enter_context(tc.tile_pool(name="res", bufs=3))

        # all input-slab DMAs issued up front; early slabs on the sync ring
        # (the scalar ring is busy with the ACT table load at t=0); const
        # DMAs slot in behind the first two slab issues
        slabs = []
        n_sync = (len(dma_sched) + 1) // 2
        for gi, g_tiles in enumerate(dma_sched):
            gb = g_tiles * P
            base = dma_offs[gi] * P * 4
            ht_sb = xp.tile([P, 4, gb], FP16, tag="ht")
            eng = nc.sync if gi < n_sync else nc.scalar
            eng.dma_start(
                ht_sb,
                ht_d[base * P:(base + 4 * gb) * P].rearrange(
                    "(p k b) -> p k b", p=P, k=4),
            )
            slabs.append(ht_sb)
            if gi == 1:
                nc.sync.dma_start(whl, whl_d[:])
                nc.sync.dma_start(bia, bi_d[:])
            elif gi == 2:
                nc.sync.dma_start(t8, t8_d[:])
                nc.sync.dma_start(t0, t0_d[:])

        def ht_chunk(t, k):
            gi = 0
            while dma_offs[gi + 1] <= t:
                gi += 1
            bs = (t - dma_offs[gi]) * P
            return slabs[gi][:, k, bs:bs + P]

        state = [None] * n_sup  # per-super (sg, vv) for stage B

        def emit_mm(si):
            """angles in turns (whl pre-scaled by 1/2pi): ang[p,g,w]"""
            sg = supers[si]
            ang = angp.tile([P, sg, 4], F32)
            for lt in range(sg):
                for k in range(4):
                    nc.tensor.matmul(
                        ang[:, lt, :],
                        ht_chunk(sup_offs[si] + lt, k),
                        whl[:, 4 * k:4 * k + 4],
                        start=(k == 0), stop=(k == 3),
                    )
            return ang

        def emit_A(si, ang):
            """angles -> pair tables' sin/cos slots (fp16)."""
            sg = supers[si]
            # m = th in turns + biad (cos row gets +1/4 turn)
            m = scr.tile([P, sg, 2, 4], F32, tag="m")
            nc.vector.tensor_add(
                m, ang.unsqueeze(2).broadcast_to([P, sg, 2, 4]),
                bia.unsqueeze(1).broadcast_to([P, sg, 2, 4]),
            )
            # r = rint(m) via the magic lattice, s = m - r in [-0.5, 0.5]
            # (kept on DVE: these are latency-critical, GpSimd TS is ~7x slower)
            r = scr.tile([P, sg, 2, 4], F32, tag="r")
            nc.vector.tensor_scalar(
                r, m, MAGIC, MAGIC,
                op0=mybir.AluOpType.add, op1=mybir.AluOpType.subtract,
            )
            s = scr.tile([P, sg, 2, 4], F32, tag="s")
            nc.vector.tensor_sub(s, m, r)
            # two ACTs write w-major straight into the pair tables' first 4
            # slots: wb[., X, 0:4] = [s_hi, c_hi, s_lo, c_lo] for pair X
            wb = stg.tile([P, sg, 2, 8], FP16, tag="wb")
            for X in (0, 1):
                nc.scalar.activation(
                    wb[:, :, X, 0:4].rearrange("p g (w r) -> p g w r", w=2),
                    s[:, :, :, 2 * X:2 * X + 2].rearrange("p g r w -> p g w r"),
                    mybir.ActivationFunctionType.Sin, scale=2.0 * PI,
                )
            state[si] = wb

        def emit_B(si):
            """pair tables + contraction for super si."""
            sg = supers[si]
            wb = state[si]
            # product slots 4:8 = [s_hi; c_hi] x [s_lo, c_lo]
            for X in (0, 1):
                nc.vector.tensor_mul(
                    wb[:, :, X, 4:8].rearrange("p g (a b) -> p g a b", a=2),
                    wb[:, :, X, 0:2].unsqueeze(3).broadcast_to([P, sg, 2, 2]),
                    wb[:, :, X, 2:4].unsqueeze(2).broadcast_to([P, sg, 2, 2]),
                )
            w01 = wb[:, :, 0, :]
            w23 = wb[:, :, 1, :]
            # tq[p,g,(k a),b] = T8[(k a), b] * w23[b]; 2x tree-add over b
            tq = tqp.tile([P, sg, 18, 8], FP16, tag="tq")
            nc.vector.tensor_mul(
                tq,
                w23.unsqueeze(2).broadcast_to([P, sg, 18, 8]),
                t8.unsqueeze(1).broadcast_to([P, sg, 18, 8]),
            )
            lq1 = tqp.tile([P, sg, 18, 4], FP16, tag="lq1")
            nc.vector.tensor_add(lq1, tq[:, :, :, 0:4], tq[:, :, :, 4:8])
            lq2 = tqp.tile([P, sg, 18, 2], FP16, tag="lq2")
            nc.vector.tensor_add(lq2, lq1[:, :, :, 0:2], lq1[:, :, :, 2:4])
            qk = stg.tile([P, sg, 18], FP16, tag="qk")
            nc.vector.tensor_add(qk, lq2[:, :, :, 0], lq2[:, :, :, 1])
            # qkf in fp32 so the GpSimd tail never touches fp16
            qkf = stg.tile([P, sg, 18], F32, tag="qkf")
            nc.vector.tensor_add(
                qkf, qk, t0.unsqueeze(1).broadcast_to([P, sg, 18])
            )
            qkv = qkf.rearrange("p g (k a) -> p g k a", k=2)
            sk = stg.tile([P, sg, 2, 8], F32, tag="sk")
            nc.vector.tensor_mul(
                sk, qkv[:, :, :, 0:8],
                w01.unsqueeze(2).broadcast_to([P, sg, 2, 8]),
            )
            # tree + a0 term on GpSimd (fp32), freeing the DVE queue
            m1 = scr.tile([P, sg, 2, 4], F32, tag="m1")
            nc.gpsimd.tensor_add(m1, sk[:, :, :, 0:4], sk[:, :, :, 4:8])
            m2 = scr.tile([P, sg, 2, 2], F32, tag="m2")
            nc.gpsimd.tensor_add(m2, m1[:, :, :, 0:2], m1[:, :, :, 2:4])
            m3 = scr.tile([P, sg, 2], F32, tag="m3")
            nc.gpsimd.tensor_add(m3, m2[:, :, :, 0], m2[:, :, :, 1])
            t0g = sup_offs[si]
            ro = rp.tile([P, sg, 2], F32, tag="ro")
            nc.gpsimd.tensor_add(ro, m3, qkv[:, :, :, 8])
            # stream this super's slice out now; hides the ~2us DMA
            # completion latency behind later supers' compute
            nc.sync.dma_start(out_d[:, t0g:t0g + sg, :], ro)

        # software pipeline: mm(k) | B(k-1) | A(k)
        angs = [None] * n_sup
        for si in range(n_sup):
            angs[si] = emit_mm(si)
            if si >= 1:
                emit_B(si - 1)
            emit_A(si, angs[si])
        emit_B(n_sup - 1)

    nc.finalize()
    return nc


_NC_CACHE = {}


def _get_nc(rows=ROWS):
    if rows not in _NC_CACHE:
        _NC_CACHE[rows] = build_bass(rows=rows)
    return _NC_CACHE[rows]


def _host_consts(pre_w, pre_b, q_weights, post_w, post_b):
    pre_w = np.asarray(pre_w, dtype=np.float32) / (2.0 * np.pi)
    wh = pre_w.astype(np.float16)
    # whl[f_loc, 4k + j] = W[j, 128k + f_loc] / 2pi  (fp16, "turns")
    whl = np.zeros((P, 16), dtype=np.float16)
    for k in range(4):
        whl[:, 4 * k:4 * k + 4] = wh.T[P * k:P * (k + 1)]
    T = _build_T(
        np.asarray(q_weights, np.float64),
        np.asarray(post_w, np.float64),
        np.asarray(post_b, np.float64),
    ).reshape(2, 9, 9)  # [k, a, b] in basis [1, s_lo, c_lo, s_hi, ...]
    # device slot order [s_hi, c_hi, s_lo, c_lo, shsl, shcl, chsl, chcl, 1]
    perm = [3, 6, 1, 2, 4, 5, 7, 8, 0]
    Tk = T[:, perm][:, :, perm]
    t8c = np.broadcast_to(
        Tk[:, :, 0:8].reshape(18, 8).astype(np.float16), (P, 18, 8)).copy()
    t0c = np.broadcast_to(
        Tk[:, :, 8].reshape(18).astype(np.float16), (P, 18)).copy()
    pb = np.asarray(pre_b, np.float64)
    # biad = (pre_b + [0, pi/2]) / 2pi  (sin row, cos row), in turns
    b2 = (np.stack([pb, pb + 0.5 * np.pi]) / (2.0 * np.pi)).astype(np.float32)
    biad = np.broadcast_to(b2, (P, 2, 4)).copy()
    return {
        "whl": np.ascontiguousarray(whl),
        "biad": np.ascontiguousarray(biad),
        "t8c": np.ascontiguousarray(t8c),
        "t0c": np.ascontiguousarray(t0c),
    }


def _split_transpose(x):
    """x [ROWS, F] f32 -> flat fp16: concatenated per-slab [P, 4, gb] packs,
    pack[p, k, b] = val[slab_row0 + b, 128*k + p]."""
    rows = x.shape[0]
    if rows == ROWS:
        sched = DMA_SCHED
    else:
        sched = []
        while sum(sched) < rows // P:
            sched.append(min(8, rows // P - sum(sched)))
    h = x.astype(np.float16)
    parts = []
    r0 = 0
    for s in sched:
        gb = s * P
        blk = h[r0:r0 + gb].reshape(gb, 4, P).transpose(2, 1, 0)
        parts.append(np.ascontiguousarray(blk).reshape(-1))
        r0 += gb
    return np.concatenate(parts)


def run(input_features, pre_w, pre_b, q_weights, post_w, post_b, **spmd_kwargs):
    x = np.asarray(input_features, dtype=np.float32)
    assert x.shape == (B_TOTAL, F_IN), x.shape
    consts = _host_consts(pre_w, pre_b, q_weights, post_w, post_b)
    in_maps = []
    for c in range(N_CORES):
        ht = _split_transpose(x[c * ROWS:(c + 1) * ROWS])
        in_maps.append(dict(consts, htp=ht))
    nc = _get_nc()
    r = run_bass_kernel_spmd(nc, in_maps, core_ids=list(range(N_CORES)), **spmd_kwargs)
    # out_dev[p, t, k] -> out[t*128 + p, k]
    out = np.concatenate(
        [r.results[c]["out"].transpose(1, 0, 2).reshape(ROWS, 2) for c in range(N_CORES)],
        axis=0,
    )
    return out.astype(np.float32), r


def kernel(input_features, pre_w, pre_b, q_weights, post_w, post_b):
    out, _ = run(input_features, pre_w, pre_b, q_weights, post_w, post_b)
    return out


# revision 41
# speedup vs baseline: 1.0875x; 1.0732x over previous
"""DressedQuantumNet on 8 TRN2 NeuronCores (pure data parallel).

Math: pre-net angles th = X @ pre_w.T + pre_b.  The quantum circuit after
the batch-dependent RY(th) layer is a FIXED unitary V (it only depends on
q_weights); the whole network collapses to an 81-coefficient multilinear
polynomial in v_w = [1, sin th_w, cos th_w]:

  out_k = sum_{m in 3^4} T_k[m] * prod_w v_w[m_w]

T_k is precomputed on host (tiny); the device computes the [B,512]@[512,4]
matmul, sin/cos, and the batched contraction.

v3 device layout (per core, batch on SBUF partitions):
  - X shipped fp16-only (rel err ~1.2e-3 vs the 2e-2 gate); host
    pre-transposed 1 MiB slabs; slabs alternate sync/scalar HWDGE rings.
  - 4 accumulating matmuls per 128-row tile -> angles [P, sg, 4] in PSUM.
  - per-super quantum stage, software-pipelined as  mm(k) | B(k-1) | A(k)
    so the DVE queue never head-of-line blocks on DMA:
      A(k): angles -> sin arguments in "turns" (3 ops: scalar_tensor_tensor
            + magic-number rint + sub), Sin activation with scale=2pi.
      B(k): pair tables (8-wide, no ones-row), tq = T8*w23, 2x-mode
            tree-adds instead of 1x tensor_reduce, then the w01 side.
  - all quantum-stage tensors fp16 with 4B-aligned power-of-2 layouts.
"""

from contextlib import ExitStack

import numpy as np

import concourse.bass as bass
import concourse.bacc as bacc_mod
import concourse.mybir as mybir
from concourse.bass_utils import run_bass_kernel_spmd
from concourse.tile import TileContext
from concourse.tile_rust import add_dep_helper

N_CORES = 8
B_TOTAL = 65536
F_IN = 512
ROWS = B_TOTAL // N_CORES   # 8192 rows per core
P = 128
N_TILES = ROWS // P         # 64 row-tiles

F32 = mybir.dt.float32
FP16 = mybir.dt.float16
PI = float(np.pi)
MAGIC = float(1.5 * 2 ** 23)

N_QUBITS, VAR_DEPTH = 4, 3

# DMA slab sizes (tiles) and quantum-stage super-group sizes (tiles).
DMA_SCHED = [2, 6, 8, 8, 8, 8, 8, 8, 8]
SUPERS = [2, 6, 24, 16, 8, 8]
assert sum(DMA_SCHED) == N_TILES and sum(SUPERS) == N_TILES


# ----------------------------------------------------------------- host math
def _gate_1q(g, w):
    ops = [np.eye(2, dtype=complex)] * N_QUBITS
    ops[w] = g
    U = ops[0]
    for i in range(1, N_QUBITS):
        U = np.kron(U, ops[i])
    return U


def _bit(i, w):  # wire 0 = most significant
    return (i >> (N_QUBITS - 1 - w)) & 1


def _cnot(c, t):
    M = np.zeros((16, 16), dtype=complex)
    for i in range(16):
        j = i ^ (1 << (N_QUBITS - 1 - t)) if _bit(i, c) else i
        M[j, i] = 1.0
    return M


def _ry(theta):
    c, s = np.cos(theta / 2), np.sin(theta / 2)
    return np.array([[c, -s], [s, c]], dtype=complex)


def _rz(theta):
    ph = np.exp(1j * theta / 2)
    return np.array([[np.conj(ph), 0], [0, ph]], dtype=complex)


def _fixed_unitary(qw):
    V = np.eye(16, dtype=complex)

    def app(Gm):
        nonlocal V
        V = Gm @ V

    def entangle():
        app(_cnot(0, 1)); app(_cnot(2, 3)); app(_cnot(1, 2))

    for k in range(VAR_DEPTH):
        entangle()
        for w in range(N_QUBITS):
            app(_gate_1q(_ry(qw[k, w]), w))
        for w in range(N_QUBITS):
            app(_gate_1q(_rz(qw[k, w]), w))
    for k in range(VAR_DEPTH):
        entangle()
        for w in range(N_QUBITS):
            app(_gate_1q(_ry(qw[k, w]), w))
        for w in range(N_QUBITS):
            app(_gate_1q(_rz(qw[3 + k, w]), w))
    entangle()
    return V


def _build_T(q_weights, post_w, post_b):
    """[2, 81] coefficients; post_b folded into the constant term."""
    V = _fixed_unitary(np.asarray(q_weights, dtype=np.float64))
    E = np.zeros((3, 2, 2))
    E[0] = [[0.5, 0.0], [0.0, 0.5]]
    E[1] = [[-0.5, 0.0], [0.0, 0.5]]
    E[2] = [[0.0, 0.5], [0.5, 0.0]]
    Ts = []
    for k in range(2):
        C = np.zeros((16, 16), dtype=complex)
        for w in range(N_QUBITS):
            z = np.array([1.0 - 2.0 * _bit(i, w) for i in range(16)])
            C += post_w[k, w] * (V.conj().T @ np.diag(z) @ V)
        A = C.real.reshape([2] * 8)
        T = np.einsum("abcdefgh,iae,jbf,kcg,ldh->ijkl", A, E, E, E, E)
        T = T.reshape(81).copy()
        T[0] += post_b[k]
        Ts.append(T)
    return np.stack(Ts).astype(np.float32)  # [2, 81]


# ------------------------------------------------------------- device kernel
def build_bass(rows=ROWS):
    n_tiles = rows // P
    if rows == ROWS:
        dma_sched, supers = DMA_SCHED, SUPERS
    else:
        dma_sched = []
        while sum(dma_sched) < n_tiles:
            dma_sched.append(min(8, n_tiles - sum(dma_sched)))
        supers = list(dma_sched)
    dma_offs = [0]
    for s in dma_sched:
        dma_offs.append(dma_offs[-1] + s)
    sup_offs = [0]
    for s in supers:
        sup_offs.append(sup_offs[-1] + s)
    n_sup = len(supers)

    nc = bacc_mod.Bacc(None, target_bir_lowering=False)
    # host-packed flat: concatenation of per-slab [P, 4, gb] fp16 blocks
    ht_d = nc.dram_tensor("htp", [rows * 4 * P], FP16, kind="ExternalInput")
    whl_d = nc.dram_tensor("whl", [P, 16], FP16, kind="ExternalInput")
    bi_d = nc.dram_tensor("biad", [P, 2, 4], F32, kind="ExternalInput")
    t8_d = nc.dram_tensor("t8c", [P, 18, 8], FP16, kind="ExternalInput")
    t0_d = nc.dram_tensor("t0c", [P, 18], FP16, kind="ExternalInput")
    # out_dev[p, t, k] = out[t*128 + p, k]; host unscrambles
    out_d = nc.dram_tensor("out", [P, n_tiles, 2], F32, kind="ExternalOutput")

    with TileContext(nc) as tc, ExitStack() as ctx:
        const = ctx.enter_context(tc.tile_pool(name="const", bufs=1))
        # dummy activation fed by a memset tile: forces the Sin ACT table
        # load to start immediately, overlapping the input DMA instead of
        # sitting on the first super's critical path
        wsrc = const.tile([P, 2], F32)
        nc.vector.memset(wsrc, 0.25)
        warm = const.tile([P, 2], FP16)
        nc.scalar.activation(warm, wsrc, mybir.ActivationFunctionType.Sin)
        # const tiles; DMAs interleaved into the sync ring after the first
        # slabs (see below) so slab0/1 data starts flowing first
        whl = const.tile([P, 16], FP16)
        bia = const.tile([P, 2, 4], F32)
        t8 = const.tile([P, 18, 8], FP16)
        t0 = const.tile([P, 18], FP16)

        xp = ctx.enter_context(tc.tile_pool(name="xin", bufs=9))
        angp = ctx.enter_context(tc.tile_pool(name="angp", bufs=3, space="PSUM"))
        stg = ctx.enter_context(tc.tile_pool(name="stg", bufs=3))
        scr = ctx.enter_context(tc.tile_pool(name="scr", bufs=3))
        tqp = ctx.enter_context(tc.tile_pool(name="tq", bufs=2))
        rp = ctx.enter_context(tc.tile_pool(name="res", bufs=3))

        # all input-slab DMAs issued up front; early slabs on the sync ring
        # (the scalar ring is busy with the ACT table load at t=0); const
        # DMAs slot in behind the first two slab issues
        slabs = []
        n_sync = (len(dma_sched) + 1) // 2
        for gi, g_tiles in enumerate(dma_sched):
            gb = g_tiles * P
            base = dma_offs[gi] * P * 4
            ht_sb = xp.tile([P, 4, gb], FP16, tag="ht")
            eng = nc.sync if gi < n_sync else nc.scalar
            eng.dma_start(
                ht_sb,
                ht_d[base * P:(base + 4 * gb) * P].rearrange(
                    "(p k b) -> p k b", p=P, k=4),
            )
            slabs.append(ht_sb)
            if gi == 1:
                nc.sync.dma_start(whl, whl_d[:])
                nc.sync.dma_start(bia, bi_d[:])
            elif gi == 2:
                nc.sync.dma_start(t8, t8_d[:])
                nc.sync.dma_start(t0, t0_d[:])

        def ht_chunk(t, k):
            gi = 0
            while dma_offs[gi + 1] <= t:
                gi += 1
            bs = (t - dma_offs[gi]) * P
            return slabs[gi][:, k, bs:bs + P]

        state = [None] * n_sup  # per-super wb tile for stage B
        # pipeline-order anchors: the scheduler is a sim-time list scheduler
        # that underestimates DMA latency; explicit edges stop it from
        # pinning DMA-gated A(k+1) ops ahead of ready B(k) work per queue
        last_dve = [None]  # last DVE op of the previous B chain
        last_gp = [None]   # last GpSimd op of the previous super
        last_act = [None]  # last ACT of the previous super

        def emit_mm(si):
            """angles in turns (whl pre-scaled by 1/2pi): ang[p,g,w]"""
            sg = supers[si]
            ang = angp.tile([P, sg, 4], F32)
            for lt in range(sg):
                for k in range(4):
                    nc.tensor.matmul(
                        ang[:, lt, :],
                        ht_chunk(sup_offs[si] + lt, k),
                        whl[:, 4 * k:4 * k + 4],
                        start=(k == 0), stop=(k == 3),
                    )
            return ang

        def emit_A(si, ang):
            """angles -> pair tables' sin/cos slots (fp16)."""
            sg = supers[si]
            # m = th in turns + biad (cos row gets +1/4 turn)
            m = scr.tile([P, sg, 2, 4], F32, tag="m")
            mi = nc.vector.tensor_add(
                m, ang.unsqueeze(2).broadcast_to([P, sg, 2, 4]),
                bia.unsqueeze(1).broadcast_to([P, sg, 2, 4]),
            )
            if last_dve[0] is not None:
                add_dep_helper(mi.ins, last_dve[0].ins, sync=False,
                               reason="keep DVE queue in pipeline order")
            # r = rint(m) via the magic lattice, s = m - r in [-0.5, 0.5]
            # (kept on DVE: these are latency-critical, GpSimd TS is ~7x slower)
            r = scr.tile([P, sg, 2, 4], F32, tag="r")
            nc.vector.tensor_scalar(
                r, m, MAGIC, MAGIC,
                op0=mybir.AluOpType.add, op1=mybir.AluOpType.subtract,
            )
            s = scr.tile([P, sg, 2, 4], F32, tag="s")
            nc.vector.tensor_sub(s, m, r)
            # two ACTs write w-major straight into the pair tables' first 4
            # slots: wb[., X, 0:4] = [s_hi, c_hi, s_lo, c_lo] for pair X
            wb = stg.tile([P, sg, 2, 8], FP16, tag="wb")
            for X in (0, 1):
                ai = nc.scalar.activation(
                    wb[:, :, X, 0:4].rearrange("p g (w r) -> p g w r", w=2),
                    s[:, :, :, 2 * X:2 * X + 2].rearrange("p g r w -> p g w r"),
                    mybir.ActivationFunctionType.Sin, scale=2.0 * PI,
                )
                if last_act[0] is not None:
                    add_dep_helper(ai.ins, last_act[0].ins, sync=False,
                                   reason="keep ACT queue in pipeline order")
                last_act[0] = ai
            state[si] = wb

        def emit_B(si):
            """pair tables + contraction for super si."""
            sg = supers[si]
            wb = state[si]
            # product slots 4:8 = [s_hi; c_hi] x [s_lo, c_lo]
            for X in (0, 1):
                nc.vector.tensor_mul(
                    wb[:, :, X, 4:8].rearrange("p g (a b) -> p g a b", a=2),
                    wb[:, :, X, 0:2].unsqueeze(3).broadcast_to([P, sg, 2, 2]),
                    wb[:, :, X, 2:4].unsqueeze(2).broadcast_to([P, sg, 2, 2]),
                )
            w01 = wb[:, :, 0, :]
            w23 = wb[:, :, 1, :]
            # tq[p,g,(k a),b] = T8[(k a), b] * w23[b]; 2x tree-add over b
            tq = tqp.tile([P, sg, 18, 8], FP16, tag="tq")
            nc.vector.tensor_mul(
                tq,
                w23.unsqueeze(2).broadcast_to([P, sg, 18, 8]),
                t8.unsqueeze(1).broadcast_to([P, sg, 18, 8]),
            )
            lq1 = tqp.tile([P, sg, 18, 4], FP16, tag="lq1")
            nc.vector.tensor_add(lq1, tq[:, :, :, 0:4], tq[:, :, :, 4:8])
            lq2 = tqp.tile([P, sg, 18, 2], FP16, tag="lq2")
            nc.vector.tensor_add(lq2, lq1[:, :, :, 0:2], lq1[:, :, :, 2:4])
            qk = stg.tile([P, sg, 18], FP16, tag="qk")
            nc.vector.tensor_add(qk, lq2[:, :, :, 0], lq2[:, :, :, 1])
            # qkf in fp32 so the GpSimd tail never touches fp16
            qkf = stg.tile([P, sg, 18], F32, tag="qkf")
            nc.vector.tensor_add(
                qkf, qk, t0.unsqueeze(1).broadcast_to([P, sg, 18])
            )
            qkv = qkf.rearrange("p g (k a) -> p g k a", k=2)
            sk = stg.tile([P, sg, 2, 8], F32, tag="sk")
            last_dve[0] = nc.vector.tensor_mul(
                sk, qkv[:, :, :, 0:8],
                w01.unsqueeze(2).broadcast_to([P, sg, 2, 8]),
            )
            # tree + a0 term on GpSimd (fp32), freeing the DVE queue
            m1 = scr.tile([P, sg, 2, 4], F32, tag="m1")
            g1 = nc.gpsimd.tensor_add(m1, sk[:, :, :, 0:4], sk[:, :, :, 4:8])
            if last_gp[0] is not None:
                add_dep_helper(g1.ins, last_gp[0].ins, sync=False,
                               reason="keep GpSimd queue in pipeline order")
            m2 = scr.tile([P, sg, 2, 2], F32, tag="m2")
            nc.gpsimd.tensor_add(m2, m1[:, :, :, 0:2], m1[:, :, :, 2:4])
            m3 = scr.tile([P, sg, 2], F32, tag="m3")
            nc.gpsimd.tensor_add(m3, m2[:, :, :, 0], m2[:, :, :, 1])
            t0g = sup_offs[si]
            ro = rp.tile([P, sg, 2], F32, tag="ro")
            last_gp[0] = nc.gpsimd.tensor_add(ro, m3, qkv[:, :, :, 8])
            # stream this super's slice out now; hides the ~2us DMA
            # completion latency behind later supers' compute
            nc.sync.dma_start(out_d[:, t0g:t0g + sg, :], ro)

        # software pipeline: mm(k) | B(k-1) | A(k)
        angs = [None] * n_sup
        for si in range(n_sup):
            angs[si] = emit_mm(si)
            if si >= 1:
                emit_B(si - 1)
            emit_A(si, angs[si])
        emit_B(n_sup - 1)

    nc.finalize()
    return nc


_NC_CACHE = {}


def _get_nc(rows=ROWS):
    if rows not in _NC_CACHE:
        _NC_CACHE[rows] = build_bass(rows=rows)
    return _NC_CACHE[rows]


def _host_consts(pre_w, pre_b, q_weights, post_w, post_b):
    pre_w = np.asarray(pre_w, dtype=np.float32) / (2.0 * np.pi)
    wh = pre_w.astype(np.float16)
    # whl[f_loc, 4k + j] = W[j, 128k + f_loc] / 2pi  (fp16, "turns")
    whl = np.zeros((P, 16), dtype=np.float16)
    for k in range(4):
        whl[:, 4 * k:4 * k + 4] = wh.T[P * k:P * (k + 1)]
    T = _build_T(
        np.asarray(q_weights, np.float64),
        np.asarray(post_w, np.float64),
        np.asarray(post_b, np.float64),
    ).reshape(2, 9, 9)  # [k, a, b] in basis [1, s_lo, c_lo, s_hi, ...]
    # device slot order [s_hi, c_hi, s_lo, c_lo, shsl, shcl, chsl, chcl, 1]
    perm = [3, 6, 1, 2, 4, 5, 7, 8, 0]
    Tk = T[:, perm][:, :, perm]
    t8c = np.broadcast_to(
        Tk[:, :, 0:8].reshape(18, 8).astype(np.float16), (P, 18, 8)).copy()
    t0c = np.broadcast_to(
        Tk[:, :, 8].reshape(18).astype(np.float16), (P, 18)).copy()
    pb = np.asarray(pre_b, np.float64)
    # biad = (pre_b + [0, pi/2]) / 2pi  (sin row, cos row), in turns
    b2 = (np.stack([pb, pb + 0.5 * np.pi]) / (2.0 * np.pi)).astype(np.float32)
    biad = np.broadcast_to(b2, (P, 2, 4)).copy()
    return {
        "whl": np.ascontiguousarray(whl),
        "biad": np.ascontiguousarray(biad),
        "t8c": np.ascontiguousarray(t8c),
        "t0c": np.ascontiguousarray(t0c),
    }


def _split_transpose(x):
    """x [ROWS, F] f32 -> flat fp16: concatenated per-slab [P, 4, gb] packs,
    pack[p, k, b] = val[slab_row0 + b, 128*k + p]."""
    rows = x.shape[0]
    if rows == ROWS:
        sched = DMA_SCHED
    else:
        sched = []
        while sum(sched) < rows // P:
            sched.append(min(8, rows // P - sum(sched)))
    h = x.astype(np.float16)
    parts = []
    r0 = 0
    for s in sched:
        gb = s * P
        blk = h[r0:r0 + gb].reshape(gb, 4, P).transpose(2, 1, 0)
        parts.append(np.ascontiguousarray(blk).reshape(-1))
        r0 += gb
    return np.concatenate(parts)


def run(input_features, pre_w, pre_b, q_weights, post_w, post_b, **spmd_kwargs):
    x = np.asarray(input_features, dtype=np.float32)
    assert x.shape == (B_TOTAL, F_IN), x.shape
    consts = _host_consts(pre_w, pre_b, q_weights, post_w, post_b)
    in_maps = []
    for c in range(N_CORES):
        ht = _split_transpose(x[c * ROWS:(c + 1) * ROWS])
        in_maps.append(dict(consts, htp=ht))
    nc = _get_nc()
    r = run_bass_kernel_spmd(nc, in_maps, core_ids=list(range(N_CORES)), **spmd_kwargs)
    # out_dev[p, t, k] -> out[t*128 + p, k]
    out = np.concatenate(
        [r.results[c]["out"].transpose(1, 0, 2).reshape(ROWS, 2) for c in range(N_CORES)],
        axis=0,
    )
    return out.astype(np.float32), r


def kernel(input_features, pre_w, pre_b, q_weights, post_w, post_b):
    out, _ = run(input_features, pre_w, pre_b, q_weights, post_w, post_b)
    return out
